# revision 6
# baseline (speedup 1.0000x reference)
"""GATRouter (2-layer GATv2 + actor/critic heads) on 8 Trainium2 NeuronCores.

Sharding: edges partitioned by dst-node range (6250 nodes per core), so all
segment-softmax statistics are core-local.  Per-edge features are gathered
from fp16 node tables in DRAM via dma_gather.  The segment softmax is
computed in unnormalized form (num/den), which removes the segment-max pass
entirely (attention logits are small for this model; exp never overflows
fp16).  Aggregation uses one-hot matmuls on the tensor engine with PSUM
accumulation.  Layer-1 output is exchanged with an AllGather, after which
each core rebuilds the full layer-2 gather tables locally.
"""
import sys
from contextlib import ExitStack

import numpy as np

sys.path.insert(0, "/opt/trn_rl_repo")

import concourse.bass as bass          # noqa: E402
import concourse.tile as tile          # noqa: E402
from concourse import bacc, mybir      # noqa: E402
from concourse.bass_utils import run_bass_kernel_spmd  # noqa: E402

F16 = mybir.dt.float16
F32 = mybir.dt.float32
I16 = mybir.dt.int16
AF = mybir.ActivationFunctionType
OP = mybir.AluOpType

N = 50000
E = 800000
IN = 16
H = 4
CH = 64
HID = 256
SLOPE = 0.2
NC = 8
NPB = N // NC      # 6250 nodes per core
NB = 49            # dst blocks of 128 per core
NPAD = NB * 128    # 6272
NTAB = NC * NPAD   # 50176
HALFV = NTAB // 2  # 25088 < 32768 -> int16 gather indices fit
NSB = 7            # super-blocks per core
SBB = 7            # blocks per super-block
PAD_DST = 999.0

_cache = {}


def _roundup(x, m):
    return (x + m - 1) // m * m


def _wrap16(a):
    n = len(a)
    o = a.reshape(n // 16, 16).T.astype(np.int16)
    return np.tile(o, (8, 1))


def _preprocess(edge_index):
    src = np.concatenate([edge_index[0], np.arange(N, dtype=np.int64)]).astype(np.int64)
    dst = np.concatenate([edge_index[1], np.arange(N, dtype=np.int64)]).astype(np.int64)
    core = dst // NPB
    ldst = dst - core * NPB
    blk = ldst >> 7
    dloc = ldst & 127
    srcp = (src // NPB) * NPAD + (src % NPB)
    half = (srcp >= HALFV).astype(np.int64)

    key = (core * NB + blk) * 2 + half
    cnt = np.bincount(key, minlength=NC * NB * 2)
    mpad = _roundup(int(cnt.max()), 256)
    ep = NB * 2 * mpad

    order = np.argsort(key, kind="stable")
    seg_off = np.zeros(NC * NB * 2 + 1, np.int64)
    np.cumsum(cnt, out=seg_off[1:])

    idxl = np.zeros((NC, ep), np.int64)
    idxr = np.zeros((NC, ep), np.int64)
    dlc = np.full((NC, ep), PAD_DST, np.float32)
    for c in range(NC):
        pos = 0
        for s in range(NSB):
            for hf in range(2):
                for j in range(SBB):
                    b = s * SBB + j
                    k = (c * NB + b) * 2 + hf
                    sel = order[seg_off[k]:seg_off[k + 1]]
                    n = len(sel)
                    idxl[c, pos:pos + n] = srcp[sel] - hf * HALFV
                    idxr[c, pos:pos + n] = ldst[sel]
                    dlc[c, pos:pos + n] = dloc[sel]
                    pos += mpad
    return mpad, ep, idxl, idxr, dlc


def _mk_ap(t, dims, extra_offset=0):
    """Manual AP: keep partition dim, then explicit (stride, size) free dims.
    Offsets/strides are in elements."""
    ap = t if isinstance(t, bass.AP) else t[:]
    return bass.AP(ap.tensor, ap.offset + extra_offset,
                   [list(ap.ap[0])] + [list(d) for d in dims])


def _build(mpad):
    seg_units = mpad // 128
    units_half = SBB * seg_units
    cu = units_half // 2               # units per chunk
    ce = cu * 128                      # edges per chunk
    ep = NB * 2 * mpad

    nc = bacc.Bacc("TRN2", target_bir_lowering=False, debug=False, num_devices=NC)

    xT_in = nc.dram_tensor("xT", (IN, NTAB), F16, kind="ExternalInput")
    xTloc_in = nc.dram_tensor("xTloc", (IN, NPAD), F16, kind="ExternalInput")
    W1_in = nc.dram_tensor("W1", (IN, 2 * HID), F16, kind="ExternalInput")
    b1_in = nc.dram_tensor("b1", (128, 2 * HID), F32, kind="ExternalInput")
    W2_in = nc.dram_tensor("W2", (2, 128, 2 * HID), F16, kind="ExternalInput")
    b2_in = nc.dram_tensor("b2", (128, 2 * HID), F32, kind="ExternalInput")
    att_in = nc.dram_tensor("attB", (2, 128, 2 * H), F16, kind="ExternalInput")
    cb_in = nc.dram_tensor("cbR", (2, 128, HID), F32, kind="ExternalInput")
    gb_in = nc.dram_tensor("gbR", (2, 2, 128, HID), F32, kind="ExternalInput")
    hW_in = nc.dram_tensor("headW", (2, 128, 128), F16, kind="ExternalInput")
    hb_in = nc.dram_tensor("headb", (128, 128), F32, kind="ExternalInput")
    hW2_in = nc.dram_tensor("headW2", (128, 2), F16, kind="ExternalInput")
    il_in = nc.dram_tensor("idxl", (128, ep // 16), I16, kind="ExternalInput")
    ir_in = nc.dram_tensor("idxr", (128, ep // 16), I16, kind="ExternalInput")
    dl_in = nc.dram_tensor("dstloc", (128, ep // 128), F16, kind="ExternalInput")
    iota_in = nc.dram_tensor("iota", (128, 128), F16, kind="ExternalInput")
    id_in = nc.dram_tensor("ident", (128, 128), F16, kind="ExternalInput")

    out_nodes = nc.dram_tensor("out_nodes", (128, NB, 2), F32, kind="ExternalOutput")

    tabl = [nc.dram_tensor(f"tab{l}l", (NTAB, HID), F16) for l in range(2)]
    tabr = [nc.dram_tensor(f"tab{l}r", (NPAD, HID), F16) for l in range(2)]
    ag_in = nc.dram_tensor("ag_in", (128, 2, NPAD), F16)
    ag_out = nc.dram_tensor("ag_out", (NC, 128, 2, NPAD), F16)

    with tile.TileContext(nc) as tc, ExitStack() as ctx:
        cpool = ctx.enter_context(tc.tile_pool(name="consts", bufs=1))
        gpool = ctx.enter_context(tc.tile_pool(name="gather", bufs=2))
        upool = ctx.enter_context(tc.tile_pool(name="umid", bufs=1))
        ypool = ctx.enter_context(tc.tile_pool(name="ybuf", bufs=1))
        rpool = ctx.enter_context(tc.tile_pool(name="rt", bufs=3))
        npool = ctx.enter_context(tc.tile_pool(name="numsb", bufs=2))
        fpool = ctx.enter_context(tc.tile_pool(name="fin", bufs=1))
        tpool = ctx.enter_context(tc.tile_pool(name="tabb", bufs=3))
        ipool = ctx.enter_context(tc.tile_pool(name="idxc", bufs=2))
        opool = ctx.enter_context(tc.tile_pool(name="outp", bufs=1))
        pps = ctx.enter_context(tc.tile_pool(name="ps", bufs=7, space="PSUM"))

        def ld(name, shape, dt, src_ap):
            t = cpool.tile(shape, dt, tag=name)
            nc.sync.dma_start(t[:], src_ap)
            return t

        W1_sb = ld("w1", [IN, 2 * HID], F16, W1_in[:])
        b1_sb = ld("b1", [128, 2 * HID], F32, b1_in[:])
        W2_sb = [ld(f"w2{g}", [128, 2 * HID], F16, W2_in[g]) for g in range(2)]
        b2_sb = ld("b2", [128, 2 * HID], F32, b2_in[:])
        att_sb = [ld(f"att{l}", [128, 2 * H], F16, att_in[l]) for l in range(2)]
        cb_sb = [ld(f"cb{l}", [128, HID], F32, cb_in[l]) for l in range(2)]
        g_sb = [ld(f"g{l}", [128, HID], F32, gb_in[l, 0]) for l in range(2)]
        be_sb = [ld(f"be{l}", [128, HID], F32, gb_in[l, 1]) for l in range(2)]
        hW_sb = [ld(f"hw{g}", [128, 128], F16, hW_in[g]) for g in range(2)]
        hb_sb = ld("hb", [128, 128], F32, hb_in[:])
        hW2_sb = ld("hw2", [128, 2], F16, hW2_in[:])
        iota_sb = ld("iota", [128, 128], F16, iota_in[:])
        id_sb = ld("id", [128, 128], F16, id_in[:])

        out_sb = opool.tile([128, NB, 2], F32, tag="outsb")

        def tab_write(dst_dram, t, ps, bias_ap):
            ot = tpool.tile([128, HID], F16, tag="tout")
            nc.vector.tensor_tensor(out=ot[:], in0=ps[:], in1=bias_ap, op=OP.add)
            nc.sync.dma_start(dst_dram[128 * t:128 * (t + 1), :], ot[:])

        def build_tab1():
            for t in range(NTAB // 128):
                xt = tpool.tile([IN, 128], F16, tag="xt")
                nc.sync.dma_start(xt[:], xT_in[:, 128 * t:128 * (t + 1)])
                ps = pps.tile([128, HID], F32, tag="bank")
                nc.tensor.matmul(ps[:], lhsT=xt[:], rhs=W1_sb[:, 0:HID],
                                 start=True, stop=True)
                tab_write(tabl[0], t, ps, b1_sb[:, 0:HID])
            for t in range(NB):
                xt = tpool.tile([IN, 128], F16, tag="xt")
                nc.sync.dma_start(xt[:], xTloc_in[:, 128 * t:128 * (t + 1)])
                ps = pps.tile([128, HID], F32, tag="bank")
                nc.tensor.matmul(ps[:], lhsT=xt[:],
                                 rhs=W1_sb[:, HID:2 * HID], start=True, stop=True)
                tab_write(tabr[0], t, ps, b1_sb[:, HID:2 * HID])

        def build_tab2():
            for t in range(NTAB // 128):
                sh, tb = t // NB, t % NB
                ps = pps.tile([128, HID], F32, tag="bank")
                for g in range(2):
                    lh = tpool.tile([128, 128], F16, tag="xt")
                    nc.sync.dma_start(lh[:], ag_out[sh, :, g, 128 * tb:128 * (tb + 1)])
                    nc.tensor.matmul(ps[:], lhsT=lh[:], rhs=W2_sb[g][:, 0:HID],
                                     start=(g == 0), stop=(g == 1))
                tab_write(tabl[1], t, ps, b2_sb[:, 0:HID])
            for t in range(NB):
                ps = pps.tile([128, HID], F32, tag="bank")
                for g in range(2):
                    lh = tpool.tile([128, 128], F16, tag="xt")
                    nc.sync.dma_start(lh[:], ag_in[:, g, 128 * t:128 * (t + 1)])
                    nc.tensor.matmul(ps[:], lhsT=lh[:],
                                     rhs=W2_sb[g][:, HID:2 * HID],
                                     start=(g == 0), stop=(g == 1))
                tab_write(tabr[1], t, ps, b2_sb[:, HID:2 * HID])

        def heads(b, hT):
            ps = pps.tile([128, 128], F32, tag="bank")
            for g in range(2):
                nc.tensor.matmul(ps[:], lhsT=hT[:, g, :], rhs=hW_sb[g][:],
                                 start=(g == 0), stop=(g == 1))
            a1 = fpool.tile([128, 128], F16, tag="a1")
            nc.vector.tensor_tensor(out=a1[:], in0=ps[:], in1=hb_sb[:], op=OP.add)
            a1r = fpool.tile([128, 128], F16, tag="a1r")
            nc.scalar.activation(a1r[:], a1[:], AF.Relu)
            aT_ps = pps.tile([128, 128], F16, tag="bank")
            nc.tensor.transpose(aT_ps[:], a1r[:], id_sb[:])
            aT = fpool.tile([128, 128], F16, tag="aT")
            nc.vector.tensor_copy(aT[:], aT_ps[:])
            ps2 = pps.tile([128, 2], F32, tag="bank")
            nc.tensor.matmul(ps2[:], lhsT=aT[:], rhs=hW2_sb[:], start=True, stop=True)
            nc.vector.tensor_copy(out_sb[:, b, :], ps2[:])

        def finalize(l, b, nf):
            den = fpool.tile([128, H], F32, tag="den")
            nc.vector.tensor_scalar(out=den[:], in0=nf[:, HID:HID + H],
                                    scalar1=1e-16, scalar2=None, op0=OP.add)
            rec = fpool.tile([128, H], F32, tag="rec")
            nc.vector.reciprocal(rec[:], den[:])
            h0 = fpool.tile([128, HID], F32, tag="h0")
            nc.vector.tensor_tensor(out=_mk_ap(h0, [(CH, H), (1, CH)]),
                                    in0=_mk_ap(nf, [(CH, H), (1, CH)]),
                                    in1=rec[:].to_broadcast([128, H, CH]),
                                    op=OP.mult)
            h0b = fpool.tile([128, HID], F32, tag="h0b")
            nc.vector.tensor_tensor(out=h0b[:], in0=h0[:], in1=cb_sb[l][:], op=OP.add)
            mus = fpool.tile([128, 1], F32, tag="mus")
            nc.vector.tensor_reduce(mus[:], h0b[:], axis=mybir.AxisListType.X, op=OP.add)
            mu = fpool.tile([128, 1], F32, tag="mu")
            nc.vector.tensor_scalar(out=mu[:], in0=mus[:], scalar1=1.0 / HID,
                                    scalar2=None, op0=OP.mult)
            hc = fpool.tile([128, HID], F32, tag="hc")
            nc.vector.tensor_scalar(out=hc[:], in0=h0b[:], scalar1=mu[:],
                                    scalar2=None, op0=OP.subtract)
            sq = fpool.tile([128, HID], F32, tag="sq")
            vs = fpool.tile([128, 1], F32, tag="vs")
            nc.scalar.activation(sq[:], hc[:], AF.Square, accum_out=vs[:])
            var = fpool.tile([128, 1], F32, tag="var")
            nc.vector.tensor_scalar(out=var[:], in0=vs[:], scalar1=1.0 / HID,
                                    scalar2=1e-5, op0=OP.mult, op1=OP.add)
            std = fpool.tile([128, 1], F32, tag="std")
            nc.scalar.activation(std[:], var[:], AF.Sqrt)
            rstd = fpool.tile([128, 1], F32, tag="rstd")
            nc.vector.reciprocal(rstd[:], std[:])
            hn = fpool.tile([128, HID], F32, tag="hn")
            nc.vector.tensor_scalar(out=hn[:], in0=hc[:], scalar1=rstd[:],
                                    scalar2=None, op0=OP.mult)
            hg = fpool.tile([128, HID], F32, tag="hg")
            nc.vector.tensor_tensor(out=hg[:], in0=hn[:], in1=g_sb[l][:], op=OP.mult)
            hgb = fpool.tile([128, HID], F32, tag="hgb")
            nc.vector.tensor_tensor(out=hgb[:], in0=hg[:], in1=be_sb[l][:], op=OP.add)
            pos = fpool.tile([128, HID], F32, tag="pos")
            nc.scalar.activation(pos[:], hgb[:], AF.Relu)
            neg = fpool.tile([128, HID], F32, tag="neg")
            nc.vector.tensor_scalar(out=neg[:], in0=hgb[:], scalar1=0.0,
                                    scalar2=None, op0=OP.min)
            ex = fpool.tile([128, HID], F32, tag="ex")
            nc.scalar.activation(ex[:], neg[:], AF.Exp)
            hL = fpool.tile([128, HID], F16, tag="hL")
            nc.vector.tensor_tensor(out=hL[:], in0=pos[:], in1=ex[:], op=OP.add)
            hT_ps = pps.tile([128, 2, 128], F16, tag="bank")
            for g in range(2):
                nc.tensor.transpose(hT_ps[:, g, :], hL[:, 128 * g:128 * (g + 1)],
                                    id_sb[:])
            hT = fpool.tile([128, 2, 128], F16, tag="hT")
            nc.vector.tensor_copy(hT[:], hT_ps[:])
            if l == 0:
                nc.sync.dma_start(ag_in[:, :, 128 * b:128 * (b + 1)], hT[:])
            else:
                heads(b, hT)

        def edge_layer(l):
            for s in range(NSB):
                atile = npool.tile([128, SBB, HID + H], F32, tag="nsb")
                for hf in range(2):
                    cur = {}
                    for cp in range(2):
                        u0 = (s * 2 + hf) * units_half + cp * cu
                        p0 = u0 * 128
                        ilc = ipool.tile([128, ce // 16], I16, tag="ilc")
                        nc.sync.dma_start(ilc[:], il_in[:, p0 // 16:(p0 + ce) // 16])
                        irc = ipool.tile([128, ce // 16], I16, tag="irc")
                        nc.sync.dma_start(irc[:], ir_in[:, p0 // 16:(p0 + ce) // 16])
                        dlc_t = ipool.tile([128, cu], F16, tag="dlct")
                        nc.sync.dma_start(dlc_t[:], dl_in[:, u0:u0 + cu])
                        xl = gpool.tile([128, cu, HID], F16, tag="xl")
                        half_ap = tabl[l][hf * HALFV:(hf + 1) * HALFV, :]
                        nc.gpsimd.dma_gather(
                            out_ap=xl[:], in_ap=half_ap,
                            idxs_ap=ilc[:],
                            num_idxs=ce, num_idxs_reg=ce, elem_size=HID,
                            single_packet=False)
                        xr = gpool.tile([128, cu, HID], F16, tag="xr")
                        nc.gpsimd.dma_gather(
                            out_ap=xr[:], in_ap=tabr[l][:],
                            idxs_ap=irc[:],
                            num_idxs=ce, num_idxs_reg=ce, elem_size=HID,
                            single_packet=False)
                        u = upool.tile([128, cu, HID], F16, tag="u")
                        nc.vector.tensor_tensor(out=u[:], in0=xl[:], in1=xr[:],
                                                op=OP.add)
                        r = upool.tile([128, cu, HID], F16, tag="r")
                        nc.scalar.activation(r[:], u[:], AF.Prelu, alpha=SLOPE)
                        oh = ypool.tile([128, cu, 128], F16, tag="oh")
                        nc.vector.tensor_tensor(
                            out=oh[:],
                            in0=dlc_t[:].to_broadcast([128, cu, 128]),
                            in1=_mk_ap(iota_sb, [(0, cu), (1, 128)]),
                            op=OP.is_equal)
                        e_ps = pps.tile([128, cu, H], F32, tag="bank")
                        for uu in range(cu):
                            rt_ps = pps.tile([128, 2, 128], F16, tag="bank")
                            for g in range(2):
                                nc.tensor.transpose(rt_ps[:, g, :],
                                                    r[:, uu, 128 * g:128 * (g + 1)],
                                                    id_sb[:])
                            rt = rpool.tile([128, 2, 128], F16, tag="rt")
                            nc.vector.tensor_copy(rt[:], rt_ps[:])
                            for g in range(2):
                                nc.tensor.matmul(e_ps[:, uu, :], lhsT=rt[:, g, :],
                                                 rhs=att_sb[l][:, g * H:(g + 1) * H],
                                                 start=(g == 0), stop=(g == 1))
                        w = ypool.tile([128, cu, H], F16, tag="w")
                        nc.scalar.activation(w[:], e_ps[:], AF.Exp)
                        Y = ypool.tile([128, cu, HID + H], F16, tag="Y")
                        nc.vector.tensor_tensor(
                            out=_mk_ap(Y, [(HID + H, cu), (CH, H), (1, CH)]),
                            in0=_mk_ap(xl, [(HID, cu), (CH, H), (1, CH)]),
                            in1=_mk_ap(w, [(H, cu), (1, H), (0, CH)]),
                            op=OP.mult)
                        nc.vector.tensor_copy(Y[:, :, HID:HID + H], w[:])
                        for uu in range(cu):
                            r2 = cp * cu + uu
                            j = r2 // seg_units
                            k = r2 % seg_units
                            b = s * SBB + j
                            if k == 0:
                                cur[j] = pps.tile([128, HID + H], F32, tag="bank", name="bps")
                            bps = cur[j]
                            nc.tensor.matmul(bps[:], lhsT=oh[:, uu, :],
                                             rhs=Y[:, uu, :],
                                             start=(k == 0),
                                             stop=(k == seg_units - 1))
                            if k == seg_units - 1:
                                if hf == 0:
                                    nc.vector.tensor_copy(atile[:, j, :], bps[:])
                                else:
                                    nf = npool.tile([128, HID + H], F32, tag="nf")
                                    nc.vector.tensor_tensor(
                                        out=nf[:], in0=bps[:], in1=atile[:, j, :],
                                        op=OP.add)
                                    finalize(l, b, nf)

        build_tab1()
        edge_layer(0)
        nc.gpsimd.collective_compute(
            "AllGather", OP.bypass,
            replica_groups=[list(range(NC))],
            ins=[ag_in[:]], outs=[ag_out[:]])
        build_tab2()
        edge_layer(1)
        nc.sync.dma_start(out_nodes[:], out_sb[:])

    nc.compile()
    return nc


def _host_arrays(inputs, ep, idxl, idxr, dlc):
    f16 = np.float16
    x = np.asarray(inputs["x"], np.float32)
    xT = np.zeros((IN, NTAB), f16)
    for c in range(NC):
        xT[:, c * NPAD:c * NPAD + NPB] = x[c * NPB:(c + 1) * NPB].T.astype(f16)

    def rep(v, dtype=np.float32):
        return np.tile(np.asarray(v, dtype)[None, :], (128, 1))

    Wl0 = np.asarray(inputs["Wl0"], np.float32)
    Wr0 = np.asarray(inputs["Wr0"], np.float32)
    Wl1 = np.asarray(inputs["Wl1"], np.float32)
    Wr1 = np.asarray(inputs["Wr1"], np.float32)
    W1 = np.concatenate([Wl0.T, Wr0.T], axis=1).astype(f16)
    b1 = rep(np.concatenate([inputs["bl0"], inputs["br0"]]))
    W2f = np.concatenate([Wl1.T, Wr1.T], axis=1).astype(f16)
    W2 = np.stack([W2f[0:128], W2f[128:256]])
    bl1a = np.asarray(inputs["bl1"], np.float32) - Wl1.sum(1)
    br1a = np.asarray(inputs["br1"], np.float32) - Wr1.sum(1)
    b2 = rep(np.concatenate([bl1a, br1a]))

    attB = np.zeros((2, 128, 2 * H), f16)
    for l in range(2):
        a = np.asarray(inputs[f"att{l}"], np.float32)
        for g in range(2):
            for p in range(128):
                c = g * 128 + p
                attB[l, p, g * H + c // CH] = a[c // CH, c % CH]
    cbR = np.stack([rep(inputs["cb0"]), rep(inputs["cb1"])])
    gbR = np.stack([np.stack([rep(inputs["g0"]), rep(inputs["b0"])]),
                    np.stack([rep(inputs["g1"]), rep(inputs["b1"])])])
    aW1 = np.asarray(inputs["aW1"], np.float32)
    vW1 = np.asarray(inputs["vW1"], np.float32)
    hWf = np.concatenate([aW1.T, vW1.T], axis=1).astype(f16)
    headW = np.stack([hWf[0:128], hWf[128:256]])
    ab1a = np.asarray(inputs["ab1"], np.float32) - aW1.sum(1)
    vb1a = np.asarray(inputs["vb1"], np.float32) - vW1.sum(1)
    headb = rep(np.concatenate([ab1a, vb1a]))
    headW2 = np.zeros((128, 2), f16)
    headW2[0:CH, 0] = np.asarray(inputs["aW2"], np.float32)[0]
    headW2[CH:128, 1] = np.asarray(inputs["vW2"], np.float32)[0]

    iota = np.tile(np.arange(128, dtype=f16), (128, 1))
    ident = np.eye(128, dtype=f16)

    in_maps = []
    for c in range(NC):
        xTloc = np.zeros((IN, NPAD), f16)
        xTloc[:, 0:NPB] = x[c * NPB:(c + 1) * NPB].T.astype(f16)
        dlw = dlc[c].reshape(ep // 128, 128).T.astype(f16)
        in_maps.append({
            "xT": xT, "xTloc": xTloc, "W1": W1, "b1": b1, "W2": W2, "b2": b2,
            "attB": attB, "cbR": cbR, "gbR": gbR, "headW": headW, "headb": headb,
            "headW2": headW2,
            "idxl": _wrap16(idxl[c]), "idxr": _wrap16(idxr[c]),
            "dstloc": dlw, "iota": iota, "ident": ident,
        })
    return in_maps


def kernel(**inputs):
    edge_index = np.asarray(inputs["edge_index"])
    mpad, ep, idxl, idxr, dlc = _preprocess(edge_index)
    if mpad not in _cache:
        _cache[mpad] = _build(mpad)
    nc = _cache[mpad]
    in_maps = _host_arrays(inputs, ep, idxl, idxr, dlc)
    res = run_bass_kernel_spmd(nc, in_maps, core_ids=list(range(NC)))

    ab2 = float(np.asarray(inputs["ab2"], np.float32)[0])
    vb2 = float(np.asarray(inputs["vb2"], np.float32)[0])
    logits = np.zeros(N, np.float32)
    values = np.zeros(N, np.float32)
    for c in range(NC):
        o = res.results[c]["out_nodes"]
        flat = o.transpose(1, 0, 2).reshape(NPAD, 2)[:NPB]
        logits[c * NPB:(c + 1) * NPB] = flat[:, 0] + ab2
        values[c * NPB:(c + 1) * NPB] = flat[:, 1] + vb2
    state = np.mean(values, keepdims=True).astype(np.float32)
    return logits, state


# revision 19
# speedup vs baseline: 1.1771x; 1.1771x over previous
"""GATRouter (2-layer GATv2 + actor/critic heads) on 8 Trainium2 NeuronCores.

Sharding: edges partitioned by dst-node range (6250 nodes per core), so all
segment-softmax statistics are core-local.  Per-edge features are gathered
from fp16 node tables in DRAM via dma_gather.  The segment softmax is
computed in unnormalized form (num/den), which removes the segment-max pass
entirely (attention logits are small for this model; exp never overflows
fp16).  Aggregation uses one-hot matmuls on the tensor engine with PSUM
accumulation.  Layer-1 output is exchanged with an AllGather, after which
each core rebuilds the full layer-2 gather tables locally.
"""
import sys
from contextlib import ExitStack

import numpy as np

sys.path.insert(0, "/opt/trn_rl_repo")

import concourse.bass as bass          # noqa: E402
import concourse.tile as tile          # noqa: E402
from concourse import bacc, mybir      # noqa: E402
from concourse.bass_utils import run_bass_kernel_spmd  # noqa: E402

F16 = mybir.dt.float16
F32 = mybir.dt.float32
I16 = mybir.dt.int16
AF = mybir.ActivationFunctionType
OP = mybir.AluOpType

N = 50000
E = 800000
IN = 16
H = 4
CH = 64
HID = 256
SLOPE = 0.2
NC = 8
NPB = N // NC      # 6250 nodes per core
NB = 49            # dst blocks of 128 per core
NPAD = NB * 128    # 6272
NTAB = NC * NPAD   # 50176
HALFV = NTAB // 2  # 25088 < 32768 -> int16 gather indices fit
NSB = 7            # super-blocks per core
SBB = 7            # blocks per super-block
PAD_DST = 999.0
G7 = 7 * 128       # table-build group: 7 node-tiles = 896 rows

_cache = {}


def _roundup(x, m):
    return (x + m - 1) // m * m


def _wrap16(a):
    n = len(a)
    o = a.reshape(n // 16, 16).T.astype(np.int16)
    return np.tile(o, (8, 1))


def _preprocess(edge_index):
    src = np.concatenate([edge_index[0], np.arange(N, dtype=np.int64)]).astype(np.int64)
    dst = np.concatenate([edge_index[1], np.arange(N, dtype=np.int64)]).astype(np.int64)
    core = dst // NPB
    ldst = dst - core * NPB
    blk = ldst >> 7
    dloc = ldst & 127
    srcp = (src // NPB) * NPAD + (src % NPB)
    half = (srcp >= HALFV).astype(np.int64)

    key = (core * NB + blk) * 2 + half
    cnt = np.bincount(key, minlength=NC * NB * 2)
    mpad = _roundup(int(cnt.max()), 256)
    ep = NB * 2 * mpad

    order = np.argsort(key, kind="stable")
    seg_off = np.zeros(NC * NB * 2 + 1, np.int64)
    np.cumsum(cnt, out=seg_off[1:])

    idxl = np.zeros((NC, ep), np.int64)
    idxr = np.zeros((NC, ep), np.int64)
    dlc = np.full((NC, ep), PAD_DST, np.float32)
    for c in range(NC):
        pos = 0
        for s in range(NSB):
            for hf in range(2):
                for j in range(SBB):
                    b = s * SBB + j
                    k = (c * NB + b) * 2 + hf
                    sel = order[seg_off[k]:seg_off[k + 1]]
                    n = len(sel)
                    idxl[c, pos:pos + n] = srcp[sel] - hf * HALFV
                    idxr[c, pos:pos + n] = ldst[sel]
                    dlc[c, pos:pos + n] = dloc[sel]
                    pos += mpad
    return mpad, ep, idxl, idxr, dlc


def _mk_ap(t, dims, extra_offset=0):
    """Manual AP: keep partition dim, then explicit (stride, size) free dims.
    Offsets/strides are in elements."""
    ap = t if isinstance(t, bass.AP) else t[:]
    return bass.AP(ap.tensor, ap.offset + extra_offset,
                   [list(ap.ap[0])] + [list(d) for d in dims])


def _dram_ap(handle, row0, nrow_tiles):
    """DRAM AP viewing rows [row0, row0+128*nrow_tiles) of a [V, HID] table
    as [128, nrow_tiles, HID] (row = 128*i + p)."""
    return bass.AP(handle, row0 * HID,
                   [[HID, 128], [128 * HID, nrow_tiles], [1, HID]])


def _build(mpad, for_sim=False):
    seg_units = mpad // 128
    units_half = SBB * seg_units
    cu = 14                            # units per chunk
    ce = cu * 128                      # edges per chunk
    ep = NB * 2 * mpad

    nc = bacc.Bacc("TRN2", target_bir_lowering=False, debug=False, num_devices=NC)

    xT_in = nc.dram_tensor("xT", (IN, NTAB), F16, kind="ExternalInput")
    xTloc_in = nc.dram_tensor("xTloc", (IN, NPAD), F16, kind="ExternalInput")
    W1_in = nc.dram_tensor("W1", (IN, 2 * HID), F16, kind="ExternalInput")
    b1_in = nc.dram_tensor("b1", (128, 2 * HID), F16, kind="ExternalInput")
    W2_in = nc.dram_tensor("W2", (2, 128, 2 * HID), F16, kind="ExternalInput")
    b2_in = nc.dram_tensor("b2", (128, 2 * HID), F16, kind="ExternalInput")
    att_in = nc.dram_tensor("attB", (2, 128, 2 * H), F16, kind="ExternalInput")
    cb_in = nc.dram_tensor("cbR", (2, 128, HID), F16, kind="ExternalInput")
    gb_in = nc.dram_tensor("gbR", (2, 2, 128, HID), F16, kind="ExternalInput")
    hW_in = nc.dram_tensor("headW", (2, 128, 128), F16, kind="ExternalInput")
    hb_in = nc.dram_tensor("headb", (128, 128), F32, kind="ExternalInput")
    hW2_in = nc.dram_tensor("headW2", (128, 2), F16, kind="ExternalInput")
    il_in = nc.dram_tensor("idxl", (128, ep // 16), I16, kind="ExternalInput")
    ir_in = nc.dram_tensor("idxr", (128, ep // 16), I16, kind="ExternalInput")
    dl_in = nc.dram_tensor("dstloc", (128, ep // 128), F16, kind="ExternalInput")
    iota_in = nc.dram_tensor("iota", (128, 128), F16, kind="ExternalInput")
    id_in = nc.dram_tensor("ident", (128, 128), F16, kind="ExternalInput")

    out_nodes = nc.dram_tensor("out_nodes", (128, NB, 2), F32, kind="ExternalOutput")

    # gather tables, split into halves so layer gathers can start earlier
    tabh = [[nc.dram_tensor(f"tab{l}h{h}", (HALFV, HID), F16) for h in range(2)]
            for l in range(2)]
    tabr = [nc.dram_tensor(f"tab{l}r", (NPAD, HID), F16) for l in range(2)]
    ag_in = nc.dram_tensor("ag_in", (128, 2, NPAD), F16)
    if for_sim:
        ag_out = nc.dram_tensor("ag_out", (NC, 128, 2, NPAD), F16,
                                kind="ExternalInput")
    else:
        ag_out = nc.dram_tensor("ag_out", (NC, 128, 2, NPAD), F16)

    with tile.TileContext(nc) as tc, ExitStack() as ctx:
        cpool = ctx.enter_context(tc.tile_pool(name="consts", bufs=1))
        gpool = ctx.enter_context(tc.tile_pool(name="gather", bufs=3))
        upool = ctx.enter_context(tc.tile_pool(name="umid", bufs=2))
        ypool = ctx.enter_context(tc.tile_pool(name="ybuf", bufs=3))
        rpool = ctx.enter_context(tc.tile_pool(name="rt", bufs=4))
        npool = ctx.enter_context(tc.tile_pool(name="numsb", bufs=2))
        fpool = ctx.enter_context(tc.tile_pool(name="fin", bufs=2))
        tpool = ctx.enter_context(tc.tile_pool(name="tabb", bufs=3))
        ipool = ctx.enter_context(tc.tile_pool(name="idxc", bufs=3))
        opool = ctx.enter_context(tc.tile_pool(name="outp", bufs=1))
        pps = ctx.enter_context(tc.tile_pool(name="ps", bufs=8, space="PSUM"))

        def ld(name, shape, dt, src_ap):
            t = cpool.tile(shape, dt, tag=name)
            nc.sync.dma_start(t[:], src_ap)
            return t

        W1_sb = ld("w1", [IN, 2 * HID], F16, W1_in[:])
        b1_sb = ld("b1", [128, 2 * HID], F16, b1_in[:])
        W2_sb = [ld(f"w2{g}", [128, 2 * HID], F16, W2_in[g]) for g in range(2)]
        b2_sb = ld("b2", [128, 2 * HID], F16, b2_in[:])
        att_sb = [ld(f"att{l}", [128, 2 * H], F16, att_in[l]) for l in range(2)]
        cb_sb = [ld(f"cb{l}", [128, HID], F16, cb_in[l]) for l in range(2)]
        g_sb = [ld(f"g{l}", [128, HID], F16, gb_in[l, 0]) for l in range(2)]
        be_sb = [ld(f"be{l}", [128, HID], F16, gb_in[l, 1]) for l in range(2)]
        hW_sb = [ld(f"hw{g}", [128, 128], F16, hW_in[g]) for g in range(2)]
        hb_sb = ld("hb", [128, 128], F32, hb_in[:])
        hW2_sb = ld("hw2", [128, 2], F16, hW2_in[:])
        iota_sb = ld("iota", [128, 128], F16, iota_in[:])
        id_sb = ld("id", [128, 128], F16, id_in[:])

        out_sb = opool.tile([128, NB, 2], F32, tag="outsb")
        ones1 = cpool.tile([1, 128], F16, tag="ones1")
        nc.vector.memset(ones1[:], 1.0)

        # ---------- table builds (groups of 7 node-tiles) ----------
        def build_group(xt_slices, rhs_lo, bias_ap, ob_tag, alt):
            ob = tpool.tile([128, SBB, HID], F16, tag=ob_tag, bufs=2, name="ob")
            for i2 in range(4):
                nt = min(2, SBB - 2 * i2)
                ps = pps.tile([128, nt * HID], F32, tag="bank", name="tps")
                for i3 in range(nt):
                    i = 2 * i2 + i3
                    lhs = xt_slices(i)
                    for gi, lh in enumerate(lhs):
                        nc.tensor.matmul(ps[:, i3 * HID:(i3 + 1) * HID],
                                         lhsT=lh, rhs=rhs_lo[gi],
                                         start=(gi == 0), stop=False)
                    # bias via K=1 ones-row matmul (accumulates into psum)
                    nc.tensor.matmul(ps[:, i3 * HID:(i3 + 1) * HID],
                                     lhsT=ones1[:], rhs=bias_ap[0:1, :],
                                     start=False, stop=True)
                dst = _mk_ap(ob, [(HID, nt), (1, HID)], 2 * i2 * HID)
                if (alt + i2) % 2 == 0:
                    nc.vector.tensor_copy(dst, ps[:])
                else:
                    nc.scalar.copy(dst, ps[:])
            return ob

        def build_tab(l):
            bias = b1_sb if l == 0 else b2_sb
            for hf in range(2):
                for g7 in range(HALFV // G7):
                    base = hf * HALFV + g7 * G7
                    if l == 0:
                        xt = tpool.tile([IN, G7], F16, tag="xt1", bufs=2, name="xt")
                        nc.sync.dma_start(xt[:], xT_in[:, base:base + G7])
                        sl = (lambda i, xt=xt: [xt[:, 128 * i:128 * (i + 1)]])
                        rhs = [W1_sb[:, 0:HID]]
                    else:
                        sh, off = base // NPAD, base % NPAD
                        xt = tpool.tile([128, 2, G7], F16, tag="xt2", bufs=2, name="xt")
                        nc.sync.dma_start(xt[:], ag_out[sh, :, :, off:off + G7])
                        sl = (lambda i, xt=xt: [xt[:, g, 128 * i:128 * (i + 1)]
                                                for g in range(2)])
                        rhs = [W2_sb[g][:, 0:HID] for g in range(2)]
                    ob = build_group(sl, rhs, bias[:, 0:HID], "ob", g7)
                    nc.sync.dma_start(_dram_ap(tabh[l][hf], g7 * G7, SBB), ob[:])
            for g7 in range(NPAD // G7):
                base = g7 * G7
                if l == 0:
                    xt = tpool.tile([IN, G7], F16, tag="xt1", bufs=2, name="xt")
                    nc.sync.dma_start(xt[:], xTloc_in[:, base:base + G7])
                    sl = (lambda i, xt=xt: [xt[:, 128 * i:128 * (i + 1)]])
                    rhs = [W1_sb[:, HID:2 * HID]]
                else:
                    xt = tpool.tile([128, 2, G7], F16, tag="xt2", bufs=2, name="xt")
                    nc.sync.dma_start(xt[:], ag_in[:, :, base:base + G7])
                    sl = (lambda i, xt=xt: [xt[:, g, 128 * i:128 * (i + 1)]
                                            for g in range(2)])
                    rhs = [W2_sb[g][:, HID:2 * HID] for g in range(2)]
                ob = build_group(sl, rhs, bias[:, HID:2 * HID], "ob", g7)
                nc.sync.dma_start(_dram_ap(tabr[l], g7 * G7, SBB), ob[:])

        # ---------- actor/critic heads per block ----------
        def heads(b, hT):
            ps = pps.tile([128, 128], F32, tag="bank", name="hps")
            for g in range(2):
                nc.tensor.matmul(ps[:], lhsT=hT[:, g, :], rhs=hW_sb[g][:],
                                 start=(g == 0), stop=(g == 1))
            a1 = fpool.tile([128, 128], F16, tag="a1")
            nc.vector.tensor_tensor(out=a1[:], in0=ps[:], in1=hb_sb[:], op=OP.add)
            a1r = fpool.tile([128, 128], F16, tag="a1r")
            nc.scalar.activation(a1r[:], a1[:], AF.Relu)
            aT_ps = pps.tile([128, 128], F16, tag="bank", name="atps")
            nc.tensor.transpose(aT_ps[:], a1r[:], id_sb[:])
            aT = fpool.tile([128, 128], F16, tag="aT")
            nc.vector.tensor_copy(aT[:], aT_ps[:])
            ps2 = pps.tile([128, 2], F32, tag="bank", name="h2ps")
            nc.tensor.matmul(ps2[:], lhsT=aT[:], rhs=hW2_sb[:], start=True, stop=True)
            nc.vector.tensor_copy(out_sb[:, b, :], ps2[:])

        # ---------- batched finalize for one super-block ----------
        def finalize_sb(l, s, bsb):
            den = fpool.tile([128, SBB, H], F32, tag="fsmall", bufs=6, name="den")
            nc.vector.tensor_scalar(out=den[:],
                                    in0=_mk_ap(bsb, [(HID + H, SBB), (1, H)], HID),
                                    scalar1=1e-16, scalar2=None, op0=OP.add)
            rec = fpool.tile([128, SBB, H], F32, tag="fsmall", bufs=6, name="rec")
            nc.vector.reciprocal(rec[:], den[:])
            h0 = fpool.tile([128, SBB, HID], F16, tag="fbig", bufs=4, name="h0")
            nc.vector.tensor_tensor(
                out=_mk_ap(h0, [(HID, SBB), (H, CH), (1, H)]),
                in0=_mk_ap(bsb, [(HID + H, SBB), (H, CH), (1, H)]),
                in1=_mk_ap(rec, [(H, SBB), (0, CH), (1, H)]),
                op=OP.mult)
            h0b = fpool.tile([128, SBB, HID], F16, tag="fbig", bufs=4, name="h0b")
            nc.vector.tensor_tensor(out=h0b[:], in0=h0[:],
                                    in1=_mk_ap(cb_sb[l], [(0, SBB), (1, HID)]),
                                    op=OP.add)
            mus = fpool.tile([128, SBB], F32, tag="fsmall", bufs=6, name="mus")
            nc.vector.tensor_reduce(mus[:], h0b[:], axis=mybir.AxisListType.X,
                                    op=OP.add)
            mu = fpool.tile([128, SBB], F16, tag="fsmall", bufs=6, name="mu")
            nc.vector.tensor_scalar(out=mu[:], in0=mus[:], scalar1=1.0 / HID,
                                    scalar2=None, op0=OP.mult)
            hc = fpool.tile([128, SBB, HID], F16, tag="fbig", bufs=4, name="hc")
            nc.vector.tensor_tensor(out=hc[:], in0=h0b[:],
                                    in1=_mk_ap(mu, [(1, SBB), (0, HID)]),
                                    op=OP.subtract)
            sq = fpool.tile([128, SBB, HID], F16, tag="fbig", bufs=4, name="sq")
            nc.vector.tensor_tensor(out=sq[:], in0=hc[:], in1=hc[:], op=OP.mult)
            vs = fpool.tile([128, SBB], F32, tag="fsmall", bufs=6, name="vs")
            nc.vector.tensor_reduce(vs[:], sq[:], axis=mybir.AxisListType.X, op=OP.add)
            var = fpool.tile([128, SBB], F32, tag="fsmall", bufs=6, name="var")
            nc.vector.tensor_scalar(out=var[:], in0=vs[:], scalar1=1.0 / HID,
                                    scalar2=1e-5, op0=OP.mult, op1=OP.add)
            std = fpool.tile([128, SBB], F32, tag="fsmall", bufs=6, name="std")
            nc.scalar.activation(std[:], var[:], AF.Sqrt)
            rstd = fpool.tile([128, SBB], F16, tag="fsmall", bufs=6, name="rstd")
            with nc.allow_low_precision(reason="per-node scalar rstd, fp16 ok"):
                nc.vector.reciprocal(rstd[:], std[:])
            hn = fpool.tile([128, SBB, HID], F16, tag="fbig", bufs=4, name="hn")
            nc.vector.tensor_tensor(out=hn[:], in0=hc[:],
                                    in1=_mk_ap(rstd, [(1, SBB), (0, HID)]),
                                    op=OP.mult)
            hg = fpool.tile([128, SBB, HID], F16, tag="fbig", bufs=4, name="hg")
            nc.vector.tensor_tensor(out=hg[:], in0=hn[:],
                                    in1=_mk_ap(g_sb[l], [(0, SBB), (1, HID)]),
                                    op=OP.mult)
            hgb = fpool.tile([128, SBB, HID], F16, tag="fbig", bufs=4, name="hgb")
            nc.vector.tensor_tensor(out=hgb[:], in0=hg[:],
                                    in1=_mk_ap(be_sb[l], [(0, SBB), (1, HID)]),
                                    op=OP.add)
            pos = fpool.tile([128, SBB, HID], F16, tag="fbig", bufs=4, name="pos")
            nc.scalar.activation(pos[:], hgb[:], AF.Relu)
            neg = fpool.tile([128, SBB, HID], F16, tag="fbig", bufs=4, name="neg")
            nc.vector.tensor_scalar(out=neg[:], in0=hgb[:], scalar1=0.0,
                                    scalar2=None, op0=OP.min)
            ex = fpool.tile([128, SBB, HID], F16, tag="fbig", bufs=4, name="ex")
            nc.scalar.activation(ex[:], neg[:], AF.Exp)
            hL = fpool.tile([128, SBB, HID], F16, tag="fbig", bufs=4, name="hL")
            nc.vector.tensor_tensor(out=hL[:], in0=pos[:], in1=ex[:], op=OP.add)
            for j in range(SBB):
                b = s * SBB + j
                hT_ps = pps.tile([128, 2, 128], F16, tag="bank", name="htps")
                for g in range(2):
                    nc.tensor.transpose(hT_ps[:, g, :],
                                        hL[:, j, 128 * g:128 * (g + 1)], id_sb[:])
                hT = fpool.tile([128, 2, 128], F16, tag="hT")
                nc.vector.tensor_copy(hT[:], hT_ps[:])
                if l == 0:
                    nc.sync.dma_start(ag_in[:, :, 128 * b:128 * (b + 1)], hT[:])
                else:
                    heads(b, hT)

        # ---------- per-edge pipeline for one layer ----------
        def edge_layer(l):
            pending = None
            for s in range(NSB):
                atile = npool.tile([128, SBB, HID + H], F32, tag="nsb", bufs=2)
                bsb = npool.tile([128, SBB, HID + H], F32, tag="bsb")
                for hf in range(2):
                    cur = {}
                    for cp in range(units_half // cu):
                        u0 = (s * 2 + hf) * units_half + cp * cu
                        p0 = u0 * 128
                        ilc = ipool.tile([128, ce // 16], I16, tag="ilc")
                        nc.sync.dma_start(ilc[:], il_in[:, p0 // 16:(p0 + ce) // 16])
                        irc = ipool.tile([128, ce // 16], I16, tag="irc")
                        nc.sync.dma_start(irc[:], ir_in[:, p0 // 16:(p0 + ce) // 16])
                        dlc_t = ipool.tile([128, cu], F16, tag="dlct")
                        nc.sync.dma_start(dlc_t[:], dl_in[:, u0:u0 + cu])
                        xl = gpool.tile([128, cu, HID], F16, tag="xl")
                        nc.gpsimd.dma_gather(
                            out_ap=xl[:], in_ap=tabh[l][hf][:],
                            idxs_ap=ilc[:],
                            num_idxs=ce, num_idxs_reg=ce, elem_size=HID,
                            single_packet=False)
                        xr = gpool.tile([128, cu, HID], F16, tag="xr")
                        nc.gpsimd.dma_gather(
                            out_ap=xr[:], in_ap=tabr[l][:],
                            idxs_ap=irc[:],
                            num_idxs=ce, num_idxs_reg=ce, elem_size=HID,
                            single_packet=False)
                        u = upool.tile([128, cu, HID], F16, tag="u")
                        nc.vector.tensor_tensor(out=u[:], in0=xl[:], in1=xr[:],
                                                op=OP.add)
                        r = upool.tile([128, cu, HID], F16, tag="r")
                        nc.scalar.activation(r[:], u[:], AF.Prelu, alpha=SLOPE)
                        oh = ypool.tile([128, cu, 128], F16, tag="oh")
                        nc.vector.tensor_tensor(
                            out=oh[:],
                            in0=dlc_t[:].to_broadcast([128, cu, 128]),
                            in1=_mk_ap(iota_sb, [(0, cu), (1, 128)]),
                            op=OP.is_equal)
                        e_ps = pps.tile([128, cu, H], F32, tag="bank")
                        for q0 in range(0, cu, 4):
                            nq = min(4, cu - q0)
                            rt_ps = pps.tile([128, nq, 2, 128], F16, tag="bank",
                                             name="rtps")
                            for qi in range(nq):
                                for g in range(2):
                                    nc.tensor.transpose(
                                        rt_ps[:, qi, g, :],
                                        r[:, q0 + qi, 128 * g:128 * (g + 1)],
                                        id_sb[:])
                            rt = rpool.tile([128, nq, 2, 128], F16, tag="rt",
                                            name="rt")
                            if (q0 // 4) % 2 == 0:
                                nc.vector.tensor_copy(rt[:], rt_ps[:])
                            else:
                                nc.scalar.copy(rt[:], rt_ps[:])
                            for qi in range(nq):
                                for g in range(2):
                                    nc.tensor.matmul(
                                        e_ps[:, q0 + qi, :],
                                        lhsT=rt[:, qi, g, :],
                                        rhs=att_sb[l][:, g * H:(g + 1) * H],
                                        start=(g == 0), stop=(g == 1))
                        w = ypool.tile([128, cu, H], F16, tag="w")
                        nc.scalar.activation(w[:], e_ps[:], AF.Exp)
                        Y = ypool.tile([128, cu, HID + H], F16, tag="Y")
                        nc.vector.tensor_tensor(
                            out=_mk_ap(Y, [(HID + H, cu), (H, CH), (1, H)]),
                            in0=_mk_ap(xl, [(HID, cu), (H, CH), (1, H)]),
                            in1=_mk_ap(w, [(H, cu), (0, CH), (1, H)]),
                            op=OP.mult)
                        nc.vector.tensor_copy(Y[:, :, HID:HID + H], w[:])
                        for uu in range(cu):
                            r2 = cp * cu + uu
                            j = r2 // seg_units
                            k = r2 % seg_units
                            if k == 0:
                                cur[j] = pps.tile([128, HID + H], F32, tag="bank",
                                                  name="bps")
                            bps = cur[j]
                            nc.tensor.matmul(bps[:], lhsT=oh[:, uu, :],
                                             rhs=Y[:, uu, :],
                                             start=(k == 0),
                                             stop=(k == seg_units - 1))
                            if k == seg_units - 1:
                                if hf == 0:
                                    nc.scalar.copy(atile[:, j, :], bps[:])
                                else:
                                    nc.vector.tensor_tensor(
                                        out=bsb[:, j, :], in0=bps[:],
                                        in1=atile[:, j, :], op=OP.add)
                if pending is not None:
                    finalize_sb(l, pending[0], pending[1])
                pending = (s, bsb)
            finalize_sb(l, pending[0], pending[1])

        build_tab(0)
        edge_layer(0)
        if not for_sim:
            nc.gpsimd.collective_compute(
                "AllGather", OP.bypass,
                replica_groups=[list(range(NC))],
                ins=[ag_in[:]], outs=[ag_out[:]])
        build_tab(1)
        edge_layer(1)
        nc.sync.dma_start(out_nodes[:], out_sb[:])

    nc.compile()
    return nc


def _host_arrays(inputs, ep, idxl, idxr, dlc):
    f16 = np.float16
    x = np.asarray(inputs["x"], np.float32)
    xT = np.zeros((IN, NTAB), f16)
    for c in range(NC):
        xT[:, c * NPAD:c * NPAD + NPB] = x[c * NPB:(c + 1) * NPB].T.astype(f16)

    def rep(v, dtype=np.float32):
        return np.tile(np.asarray(v, dtype)[None, :], (128, 1))

    perm = np.array([(j % H) * CH + j // H for j in range(HID)])
    Wl0 = np.asarray(inputs["Wl0"], np.float32)
    Wr0 = np.asarray(inputs["Wr0"], np.float32)
    Wl1 = np.asarray(inputs["Wl1"], np.float32)
    Wr1 = np.asarray(inputs["Wr1"], np.float32)
    W1 = np.concatenate([Wl0.T[:, perm], Wr0.T[:, perm]], axis=1).astype(f16)
    b1 = rep(np.concatenate([np.asarray(inputs["bl0"])[perm],
                             np.asarray(inputs["br0"])[perm]]), f16)
    # W2 rows see permuted h1; W2 out-cols permuted too
    W2f = np.concatenate([Wl1.T[:, perm][perm, :], Wr1.T[:, perm][perm, :]],
                         axis=1).astype(f16)
    W2 = np.stack([W2f[0:128], W2f[128:256]])
    bl1a = np.asarray(inputs["bl1"], np.float32) - Wl1.sum(1)
    br1a = np.asarray(inputs["br1"], np.float32) - Wr1.sum(1)
    b2 = rep(np.concatenate([bl1a[perm], br1a[perm]]), f16)

    attB = np.zeros((2, 128, 2 * H), f16)
    for l in range(2):
        a = np.asarray(inputs[f"att{l}"], np.float32)
        for g in range(2):
            for p in range(128):
                j = g * 128 + p          # permuted channel index
                h = j % H
                cc = j // H
                attB[l, p, g * H + h] = a[h, cc]
    cbR = np.stack([rep(np.asarray(inputs["cb0"])[perm], f16),
                    rep(np.asarray(inputs["cb1"])[perm], f16)])
    gbR = np.stack([np.stack([rep(np.asarray(inputs["g0"])[perm], f16),
                              rep(np.asarray(inputs["b0"])[perm], f16)]),
                    np.stack([rep(np.asarray(inputs["g1"])[perm], f16),
                              rep(np.asarray(inputs["b1"])[perm], f16)])])
    aW1 = np.asarray(inputs["aW1"], np.float32)
    vW1 = np.asarray(inputs["vW1"], np.float32)
    hWf = np.concatenate([aW1.T[perm, :], vW1.T[perm, :]], axis=1).astype(f16)
    headW = np.stack([hWf[0:128], hWf[128:256]])
    ab1a = np.asarray(inputs["ab1"], np.float32) - aW1.sum(1)
    vb1a = np.asarray(inputs["vb1"], np.float32) - vW1.sum(1)
    headb = rep(np.concatenate([ab1a, vb1a]))
    headW2 = np.zeros((128, 2), f16)
    headW2[0:CH, 0] = np.asarray(inputs["aW2"], np.float32)[0]
    headW2[CH:128, 1] = np.asarray(inputs["vW2"], np.float32)[0]

    iota = np.tile(np.arange(128, dtype=f16), (128, 1))
    ident = np.eye(128, dtype=f16)

    in_maps = []
    for c in range(NC):
        xTloc = np.zeros((IN, NPAD), f16)
        xTloc[:, 0:NPB] = x[c * NPB:(c + 1) * NPB].T.astype(f16)
        dlw = dlc[c].reshape(ep // 128, 128).T.astype(f16)
        in_maps.append({
            "xT": xT, "xTloc": xTloc, "W1": W1, "b1": b1, "W2": W2, "b2": b2,
            "attB": attB, "cbR": cbR, "gbR": gbR, "headW": headW, "headb": headb,
            "headW2": headW2,
            "idxl": _wrap16(idxl[c]), "idxr": _wrap16(idxr[c]),
            "dstloc": dlw, "iota": iota, "ident": ident,
        })
    return in_maps


def kernel(**inputs):
    edge_index = np.asarray(inputs["edge_index"])
    mpad, ep, idxl, idxr, dlc = _preprocess(edge_index)
    if mpad not in _cache:
        _cache[mpad] = _build(mpad)
    nc = _cache[mpad]
    in_maps = _host_arrays(inputs, ep, idxl, idxr, dlc)
    res = run_bass_kernel_spmd(nc, in_maps, core_ids=list(range(NC)))

    ab2 = float(np.asarray(inputs["ab2"], np.float32)[0])
    vb2 = float(np.asarray(inputs["vb2"], np.float32)[0])
    logits = np.zeros(N, np.float32)
    values = np.zeros(N, np.float32)
    for c in range(NC):
        o = res.results[c]["out_nodes"]
        flat = o.transpose(1, 0, 2).reshape(NPAD, 2)[:NPB]
        logits[c * NPB:(c + 1) * NPB] = flat[:, 0] + ab2
        values[c * NPB:(c + 1) * NPB] = flat[:, 1] + vb2
    state = np.mean(values, keepdims=True).astype(np.float32)
    return logits, state
